# revision 54
# baseline (speedup 1.0000x reference)
"""MirrorAttention Trainium2 kernel (fp8 DoubleRow).

Data-parallel over batch B=8: one batch per NeuronCore.  Per core:
    f_a = relu(bn(Wa x)), f_v = relu(bn(Wv x_v)), f_h = relu(bn(Wv x_h))
    A_d[n, j] = exp(scale * f_q^T f_a - 4)      (rowsums via ACT accumulator;
                                                 the -4 shift keeps fp8 A in
                                                 range and cancels in softmax)
    gT_d[n, m] = (Wg_d x + bg_d) * 256/rowsum_n (fp8)
    o_d = gT_d^T A_d ;  out_d = (Wf_d/256) o_d + bf_d + x

All heavy matmuls are fp8e4 DoubleRow (2 K-tiles per instruction, 0.5
cycles/row): input convs pair the four 128-channel chunks, scores split
MID=128 over 64 partitions x 2 K-tiles (f stored [64, 2, N] via a
partition-remap SBUF DMA), the apply pairs the 18 query blocks.  The
softmax normalization (x256 rescaled for fp8 range) folds into gT;
1/256 folds into the output-conv weights host-side.

Schedule: exp on ScalarE straight out of PSUM with fused row-sum
accumulation is the bottleneck (~96us busy, near-gapless).  B1(v) hides
the f_h/g convs as PE fillers; B1(h) hides all of B2(v); gT folds run
in-stream on GPSIMD (lagged 6 blocks in B1(v) so the g-conv fillers
land first; the final block's chain on DVE to cut tail latency).
Blocks 0-3 of B1(v) compute scores directly from the [128m, cols] fp8
staging tiles (plain fp8 matmul) so the first exps don't wait for the
remap round-trip.  The B2(h) tail runs 5 concurrent applies (2 in the
freed scores-psum pools + 3 ps3 slots) with residual converts
interleaved DVE / (PE identity-matmul residual + ACT) and bf16 stores.
DMAs are few and large (each dma_start costs ~565ns of sequencer time
+900ns completion latency).
"""

import numpy as np
import ml_dtypes

import concourse.bass as bass
import concourse.mybir as mybir
import concourse.tile as tile
import bass_rust
from concourse.bass_utils import run_bass_kernel_spmd
from concourse.tile import add_dep_helper

B, C, H, W = 8, 512, 48, 48
MID = 128
N = H * W                     # 2304 tokens
NB = N // 128                 # 18 query blocks
CCH = C // 128                # 4 contraction chunks
SCALE = float(MID) ** -0.5
EPS = 1e-5
GSC = 256.0                   # fp8 range scaling for gT (1/256 inside Wf)
EXPB = -4.0                   # constant score shift: exp(scale*s + EXPB)
                              # keeps fp8 A in range; cancels in softmax
JTS = [(0, 512), (512, 512), (1024, 512), (1536, 512), (2048, 256)]
PA, PB = 1024, 1280           # scores psum piece split
SJT_A = [(0, 512), (512, 512)]
SJT_B = [(1024, 512), (1536, 512), (2048, 256)]

F32 = mybir.dt.float32
BF16 = mybir.dt.bfloat16
FP8 = mybir.dt.float8e4
NPBF = ml_dtypes.bfloat16
NPF8 = ml_dtypes.float8_e4m3
ADD = mybir.AluOpType.add
MAX = mybir.AluOpType.max
MULT = mybir.AluOpType.mult
DR = mybir.MatmulPerfMode.DoubleRow
EXP = mybir.ActivationFunctionType.Exp
IDENT = mybir.ActivationFunctionType.Identity
COPY = mybir.ActivationFunctionType.Copy


def _split_multi_waits(nc, max_waits=1):
    """walrus in this container rejects >1 sync-wait on CTRL-class
    instructions; hoist excess waits onto preceding NoOps."""
    for f in nc.m.functions:
        for bb in f.blocks:
            insts = list(bb.instructions)
            new, changed = [], False
            for inst in insts:
                si = inst.sync_info
                if si and si.on_wait and len(si.on_wait) > max_waits:
                    waits = list(si.on_wait)
                    k = 0
                    while len(waits) > max_waits:
                        chunk, waits = waits[:max_waits], waits[max_waits:]
                        nop = mybir.InstNoOp(
                            name=f"{inst.name}_waitsplit{k}", ins=[], outs=[]
                        )
                        nop.engine = inst.engine
                        nop.sync_info = bass_rust.SyncInfo(
                            on_wait=chunk, on_update=[]
                        )
                        new.append(nop)
                        k += 1
                    inst.sync_info = bass_rust.SyncInfo(
                        on_wait=waits, on_update=list(si.on_update)
                    )
                    changed = True
                new.append(inst)
            if changed:
                bb.instructions = new


DEBUG_TAPS = False
LBL = {}


def _L(inst, label):
    try:
        LBL[inst.ins.name] = label
    except Exception:
        pass
    return inst


def _build_nc():
    nc = bass.Bass()

    def din(name, shape, dt):
        return nc.declare_dram_parameter(name, shape, dt, isOutput=False)

    x8d = din("x8", [C, N], FP8)
    xv8d = din("xv8", [C, N], FP8)
    xh8d = din("xh8", [C, N], FP8)
    xrd = din("xr", [C, N], BF16)
    # 4 fp8 weight slabs (WaT, WvT, WgavT, WgahT), [p, cch, m], c = 4p+cch
    wpack8 = din("wpack8", [128, 4 * CCH * MID], FP8)
    # bf16: (Wfav/256).T, (Wfah/256).T as [m, c], + identity I128
    wpackb = din("wpackb", [128, 2 * C + 128], BF16)
    # biases: ba(1), bv(1), bfav(4), bfah(4), bgav bcast(128), bgah(128),
    # EXPB(1)
    fpack = din("fpack", [128, 267], F32)

    oh = nc.declare_dram_parameter("oh", [C, N], BF16, isOutput=True)
    ov = nc.declare_dram_parameter("ov", [C, N], BF16, isOutput=True)

    with tile.TileContext(nc, pool_alloc_mode="queue") as tc:
        with (
            tc.tile_pool(name="consts", bufs=1) as consts,
            tc.tile_pool(name="xbuf", bufs=1) as xbuf,
            tc.tile_pool(name="fbuf", bufs=1) as fbuf,
            tc.tile_pool(name="fst", bufs=1) as fst,
            tc.tile_pool(name="abuf", bufs=1) as abuf,
            tc.tile_pool(name="gbuf", bufs=1) as gbuf,
            tc.tile_pool(name="rpool", bufs=1) as rpool,
            tc.tile_pool(name="xrbuf", bufs=1) as xrbuf,
            tc.tile_pool(name="obuf", bufs=2) as obuf,
            tc.tile_pool(name="outp", bufs=2) as outp,
            tc.tile_pool(name="outh", bufs=1) as outh,
            tc.tile_pool(name="obh", bufs=1) as obh,
            tc.tile_pool(name="sA", bufs=1, space="PSUM") as sApool,
            tc.tile_pool(name="sB", bufs=1, space="PSUM") as sBpool,
            tc.tile_pool(name="ps3", bufs=3, space="PSUM") as ps3,
        ):
            # ---- constants (SP queue, one DMA each) ----
            wp8 = consts.tile([128, 4, CCH, MID], FP8, tag="wp8")
            nc.sync.dma_start(
                out=wp8,
                in_=wpack8[:].rearrange("p (w o m) -> p w o m", w=4, o=CCH))
            fp = consts.tile([128, 267], F32, tag="fpack")
            nc.scalar.dma_start(out=fp, in_=fpack[:])
            ba_sb = fp[:, 0:1]
            bv_sb = fp[:, 1:2]
            bfav_sb = fp[:, 2 : 2 + CCH]
            bfah_sb = fp[:, 6 : 6 + CCH]
            bgav_sb = fp[:, 10:138]
            bgah_sb = fp[:, 138:266]
            expb_sb = fp[:, 266:267]

            # exp-table preload for real hw (cost-model: ~free)
            warm = consts.tile([128, 1], F32, tag="warm")
            nc.vector.memset(warm, 0.0)
            nc.scalar.activation(out=warm, in_=warm, func=EXP,
                                 bias=0.0, scale=1.0)

            # ---- persistent tiles ----
            x8 = xbuf.tile([128, CCH, N], FP8, tag="x8")
            xv8 = xbuf.tile([128, CCH, N], FP8, tag="xv8")
            xh8 = xbuf.tile([128, CCH, N], FP8, tag="xh8")
            f_a = fbuf.tile([64, 2, N], FP8, tag="f_a")
            f_v = fbuf.tile([64, 2, N], FP8, tag="f_v")
            f_h = fbuf.tile([64, 2, N], FP8, tag="f_h")
            Av = abuf.tile([128, NB, N], FP8, tag="Av")
            Ah = abuf.tile([128, NB, N], FP8, tag="Ah")
            gbv = gbuf.tile([128, NB, MID], BF16, tag="gbv")
            gbh = gbuf.tile([128, NB, MID], BF16, tag="gbh")
            gTv = gbuf.tile([128, NB, MID], FP8, tag="gTv")
            gTh = gbuf.tile([128, NB, MID], FP8, tag="gTh")
            xr = xrbuf.tile([128, CCH, N], BF16, tag="xr")

            rs0v = rpool.tile([128, NB], F32, tag="rs0v")
            rs1v = rpool.tile([128, NB], F32, tag="rs1v")
            rinvv = rpool.tile([128, NB], F32, tag="rinvv")
            rs0h = rpool.tile([128, NB], F32, tag="rs0h")
            rs1h = rpool.tile([128, NB], F32, tag="rs1h")
            rinvh = rpool.tile([128, NB], F32, tag="rinvh")

            def staged(dma, after):
                if after is not None:
                    add_dep_helper(dma.ins, after.ins, sync=True,
                                   reason="stage DMA")
                return dma

            # head loads, finely chunked so the first exps start early:
            # x cols [0:512], xv [0:512], x [512:1024], x [1024:2304] x2
            nc.sync.dma_start(out=x8[:, :, 0:512], in_=x8d[:, 0:512])
            nc.sync.dma_start(out=xv8[:, :, 0:512], in_=xv8d[:, 0:512])
            nc.sync.dma_start(out=x8[:, :, 512:PA], in_=x8d[:, 512:PA])
            nc.sync.dma_start(out=x8[:, :, PA:1664], in_=x8d[:, PA:1664])
            nc.sync.dma_start(out=x8[:, :, 1664:N], in_=x8d[:, 1664:N])

            # ---- conv helpers ----
            def conv_mms(w_idx, xsb, pt, poff, jts):
                for j0, jw in jts:
                    for ci, c in enumerate((0, 2)):
                        nc.tensor.matmul(
                            pt[:, j0 - poff : j0 - poff + jw],
                            lhsT=wp8[:, w_idx, c : c + 2, :],
                            rhs=xsb[:, c : c + 2, j0 : j0 + jw],
                            start=(ci == 0), stop=(ci == 1), perf_mode=DR,
                        )

            def relu_to(stage, pt, bias_ap, lo, hi, soff=0, poff=None):
                if poff is None:
                    poff = lo
                return nc.vector.tensor_scalar(
                    out=stage[:, lo - soff : hi - soff],
                    in0=pt[:, lo - poff : hi - poff],
                    scalar1=bias_ap, scalar2=0.0, op0=ADD, op1=MAX)

            def remap(fdst, stage, lo, hi, soff=0):
                # [128, w] -> [64, 2, w]: SBUF partition-remap, m = 2p+t
                return nc.sync.dma_start(
                    out=fdst[:, :, lo:hi], in_=stage[:, lo - soff : hi - soff])

            # f_a / f_v(jt0) head convs.  The fp8 pre-remap stagings (ffa,
            # ffb, ffv0) persist and serve blocks 0-1 of B1(v) directly as
            # [128m, cols] operands (plain fp8 matmul, m on partitions), so
            # the first exps don't wait for the remap DMA round-trip.
            ffa = fst.tile([128, PA], FP8, tag="ffa")
            ffb = fst.tile([128, PB], FP8, tag="ffb")
            ffv0 = fst.tile([128, 512], FP8, tag="ffv0")

            faA = sApool.tile([128, PA], F32, tag="sA")
            conv_mms(0, x8, faA, 0, [(0, 512)])
            fv0p = ps3.tile([128, 512], F32, tag="ps3")
            conv_mms(1, xv8, fv0p, 0, JTS[:1])
            conv_mms(0, x8, faA, 0, [(512, 512)])
            # first f_a relu on ACT (idle until the first exp) so the DVE
            # chain starts with f_v's relu in parallel
            nc.scalar.activation(
                out=ffa[:, 0:512], in_=faA[:, 0:512],
                func=mybir.ActivationFunctionType.Relu,
                bias=ba_sb, scale=1.0)
            fv_relu0 = relu_to(ffv0, fv0p, bv_sb, 0, 512, poff=0)
            relu_to(ffa, faA, ba_sb, 512, PA, poff=0)
            remap(f_v, ffv0, 0, 512)
            remap(f_a, ffa, 0, PA)

            # remaining loads, staged off the early relus
            staged(nc.sync.dma_start(out=xv8[:, :, 512:N],
                                     in_=xv8d[:, 512:N]), fv_relu0)
            staged(nc.sync.dma_start(out=xh8, in_=xh8d[:]), fv_relu0)
            wpb = consts.tile([128, 2 * C + 128], BF16, tag="wpb")
            staged(nc.sync.dma_start(out=wpb, in_=wpackb[:]), fv_relu0)
            ident_sb = wpb[:, 2 * C : 2 * C + 128]

            # f_a piece B: conv into borrowed sB psum, two relu pieces
            faB = sBpool.tile([128, PB], F32, tag="sB")
            conv_mms(0, x8, faB, PA, [(1024, 512), (1536, 128)])
            relu_to(ffb, faB, ba_sb, PA, 1664, soff=PA, poff=PA)
            conv_mms(0, x8, faB, PA, [(1664, 384), (2048, 256)])
            relu_to(ffb, faB, ba_sb, 1664, N, soff=PA, poff=PA)
            remap(f_a, ffb, PA, N, soff=PA)

            # ---- B1 machinery ----
            def scores(f_q, blk, direct=None, tag=""):
                if direct is not None:
                    # staging-based: [128m, cols] operands, plain fp8 mm
                    if blk < 4:
                        q = direct[:, blk * 128 : (blk + 1) * 128]
                    else:
                        q = fstvB[:, blk * 128 - 512 : (blk + 1) * 128 - 512]
                    sA = sApool.tile([128, PA], F32, tag="sA")
                    for j0, jw in SJT_A:
                        _L(nc.tensor.matmul(
                            sA[:, j0 : j0 + jw], lhsT=q,
                            rhs=ffa[:, j0 : j0 + jw], start=True, stop=True),
                           f"dscore{tag}A b{blk}")
                    sB = sBpool.tile([128, PB], F32, tag="sB")
                    for j0, jw in SJT_B:
                        _L(nc.tensor.matmul(
                            sB[:, j0 - PA : j0 - PA + jw], lhsT=q,
                            rhs=ffb[:, j0 - PA : j0 - PA + jw],
                            start=True, stop=True), f"dscore{tag}B b{blk}")
                    return sA, sB
                q = f_q[:, :, blk * 128 : (blk + 1) * 128]
                sA = sApool.tile([128, PA], F32, tag="sA")
                for j0, jw in SJT_A:
                    _L(nc.tensor.matmul(
                        sA[:, j0 : j0 + jw], lhsT=q,
                        rhs=f_a[:, :, j0 : j0 + jw],
                        start=True, stop=True, perf_mode=DR),
                       f"score{tag}A b{blk}")
                sB = sBpool.tile([128, PB], F32, tag="sB")
                for j0, jw in SJT_B:
                    _L(nc.tensor.matmul(
                        sB[:, j0 - PA : j0 - PA + jw], lhsT=q,
                        rhs=f_a[:, :, j0 : j0 + jw],
                        start=True, stop=True, perf_mode=DR),
                       f"score{tag}B b{blk}")
                return sA, sB

            def fold_blk(gb_src, gT_dst, rinv, blk):
                _L(nc.gpsimd.tensor_scalar(
                    out=gT_dst[:, blk, :], in0=gb_src[:, blk, :],
                    scalar1=rinv[:, blk : blk + 1], scalar2=GSC,
                    op0=MULT, op1=MULT), f"fold b{blk}")

            def b1_loop(f_q, A, rs0, rs1, rinv, fold_src=None, fold_dst=None,
                        fold_lag=6, fillers=None,
                        direct_q=None, split0=None, split1=None, tag=""):
                # direct_q: staging lhsT for blocks 0-1 (head latency)
                # split0: extra accum tile; block 0's piece-A runs as two
                # sub-exps so the first exp starts before f_a is complete
                fi = 0
                dq = direct_q
                sA, sB = scores(f_q, 0, direct=dq, tag=tag)
                for blk in range(NB):
                    if blk == 0 and split0 is not None:
                        nc.scalar.activation(
                            out=A[:, 0, 0:512], in_=sA[:, 0:512], func=EXP,
                            bias=expb_sb, scale=SCALE, accum_out=rs0[:, 0:1])
                        nc.scalar.activation(
                            out=A[:, 0, 512:PA], in_=sA[:, 512:PA], func=EXP,
                            bias=expb_sb, scale=SCALE, accum_out=split0)
                    else:
                        _L(nc.scalar.activation(
                            out=A[:, blk, 0:PA], in_=sA, func=EXP,
                            bias=expb_sb, scale=SCALE,
                            accum_out=rs0[:, blk : blk + 1]), f"exp{tag}A b{blk}")
                    _L(nc.scalar.activation(
                        out=A[:, blk, PA:N], in_=sB, func=EXP,
                        bias=expb_sb, scale=SCALE,
                        accum_out=rs1[:, blk : blk + 1]),
                       f"exp{tag}B b{blk}")
                    fb = blk - fold_lag
                    if fold_dst is not None and fb >= 0:
                        b = slice(fb, fb + 1)
                        # last block: all-DVE chain (fewer cross-engine
                        # hops on the tail critical path)
                        eng = nc.vector if fb == NB - 1 else nc.gpsimd
                        if fb == 0 and split0 is not None:
                            eng.tensor_tensor(
                                out=rs0[:, b], in0=rs0[:, b], in1=split0,
                                op=ADD)
                        eng.tensor_tensor(
                            out=rs0[:, b], in0=rs0[:, b], in1=rs1[:, b],
                            op=ADD)
                        nc.vector.reciprocal(out=rinv[:, b], in_=rs0[:, b])
                        if fb == NB - 1:
                            _L(nc.vector.tensor_scalar(
                                out=fold_dst[:, fb, :],
                                in0=fold_src[:, fb, :],
                                scalar1=rinv[:, fb : fb + 1], scalar2=GSC,
                                op0=MULT, op1=MULT), f"fold b{fb} dve")
                        else:
                            fold_blk(fold_src, fold_dst, rinv, fb)
                    if blk + 1 < NB:
                        dq = direct_q if blk + 1 < 4 else None
                        sA, sB = scores(f_q, blk + 1, direct=dq, tag=tag)
                    rem = max(NB - 3 - blk, 1)
                    n_take = ((fi == 0) + (len(fillers) - fi + rem - 1)
                              // rem) if fillers else 0
                    for _ in range(n_take):
                        if fi < len(fillers):
                            fillers[fi]()
                            fi += 1
                if fillers:
                    while fi < len(fillers):
                        fillers[fi]()
                        fi += 1
                if fold_dst is not None:
                    for fb in range(max(NB - fold_lag, 0), NB):
                        b = slice(fb, fb + 1)
                        nc.gpsimd.tensor_tensor(
                            out=rs0[:, b], in0=rs0[:, b], in1=rs1[:, b],
                            op=ADD)
                        nc.vector.reciprocal(out=rinv[:, b], in_=rs0[:, b])
                        fold_blk(fold_src, fold_dst, rinv, fb)

            # ---- B2 machinery ----
            oh_t = oh.rearrange("(o p) n -> p o n", p=128)
            ov_t = ov.rearrange("(o p) n -> p o n", p=128)

            def apply_unit(gT, A, jt_i, copy_eng="dve", pool=None):
                j0, jw = JTS[jt_i]
                ap = (pool or ps3).tile(
                    [128, 512], F32, tag=(pool or ps3).name)
                for pi in range(NB // 2):
                    _L(nc.tensor.matmul(
                        ap[:, :jw],
                        lhsT=gT[:, 2 * pi : 2 * pi + 2, :],
                        rhs=A[:, 2 * pi : 2 * pi + 2, j0 : j0 + jw],
                        start=(pi == 0), stop=(pi == NB // 2 - 1),
                        perf_mode=DR), f"applyV jt{jt_i} p{pi}")
                ob = obuf.tile([128, 512], BF16, tag="obuf")
                if copy_eng == "act":
                    nc.scalar.activation(out=ob[:, :jw], in_=ap[:, :jw],
                                         func=COPY, bias=0.0, scale=1.0)
                else:
                    nc.vector.tensor_copy(out=ob[:, :jw], in_=ap[:, :jw])
                return ob

            def outconv_unit(ob, w_idx, bf_sb, out_t, jt_i, acts=()):
                # acts: c-chunk indices whose residual-add goes through
                # PE (identity matmul on xr) + ACT convert, else DVE
                j0, jw = JTS[jt_i]
                outt = outp.tile([128, CCH, 512], BF16, tag="outp")
                for co in range(CCH):
                    cp = ps3.tile([128, 512], F32, tag="ps3")
                    nc.tensor.matmul(
                        cp[:, :jw],
                        lhsT=wpb[:, w_idx * C + co * 128 :
                                 w_idx * C + co * 128 + 128],
                        rhs=ob[:, :jw], start=True,
                        stop=(co not in acts))
                    if co in acts:
                        nc.tensor.matmul(
                            cp[:, :jw], lhsT=ident_sb,
                            rhs=xr[:, co, j0 : j0 + jw],
                            start=False, stop=True)
                        nc.scalar.activation(
                            out=outt[:, co, :jw], in_=cp[:, :jw],
                            func=IDENT, bias=bf_sb[:, co : co + 1],
                            scale=1.0)
                    else:
                        nc.vector.scalar_tensor_tensor(
                            out=outt[:, co, :jw], in0=cp[:, :jw],
                            scalar=bf_sb[:, co : co + 1],
                            in1=xr[:, co, j0 : j0 + jw], op0=ADD, op1=ADD)
                nc.sync.dma_start(
                    out=out_t[:, :, j0 : j0 + jw], in_=outt[:, :, :jw])

            # ================= emission =================
            fstvB = fst.tile([128, N - 512], FP8, tag="ffvB")

            def mk_fpiece(w_idx, bias, xsb, fdst, stage_tag, jts, lo, hi,
                          stage0=None):
                def go():
                    relu = None
                    stage = (stage0 if stage0 is not None else
                             fst.tile([128, hi - lo], FP8, tag=stage_tag))
                    for j0, jw in jts:
                        pt = ps3.tile([128, 512], F32, tag="ps3")
                        conv_mms(w_idx, xsb, pt, j0, [(j0, jw)])
                        relu = relu_to(stage, pt, bias, j0, j0 + jw,
                                       soff=lo, poff=j0)
                    remap(fdst, stage, lo, hi, soff=lo)
                    return relu
                return go

            def mk_g(w_idx, bg, gb_dst, blk):
                def go():
                    gp = ps3.tile([128, 512], F32, tag="ps3")
                    for ci, c in enumerate((0, 2)):
                        nc.tensor.matmul(
                            gp[:, :MID],
                            lhsT=x8[:, c : c + 2,
                                    blk * 128 : (blk + 1) * 128],
                            rhs=wp8[:, w_idx, c : c + 2, :],
                            start=(ci == 0), stop=(ci == 1), perf_mode=DR)
                    nc.vector.tensor_tensor(
                        out=gb_dst[:, blk, :], in0=gp[:, :MID], in1=bg,
                        op=ADD)
                return go

            fh_done = []

            def mk_fh():
                def go():
                    stage = fst.tile([128, N], FP8, tag="ffh")
                    for j0, jw in JTS:
                        pt = ps3.tile([128, 512], F32, tag="ps3")
                        conv_mms(1, xh8, pt, j0, [(j0, jw)])
                        fh_done.append(
                            relu_to(stage, pt, bv_sb, j0, j0 + jw, poff=j0))
                    remap(f_h, stage, 0, N)
                return go

            fillers_v = [
                mk_fpiece(1, bv_sb, xv8, f_v, "ffvB",
                          JTS[1:], 512, N, stage0=fstvB),
            ]
            fillers_v += [mk_g(2, bgav_sb, gbv, b) for b in range(8)]
            fillers_v += [mk_fh()]
            fillers_v += [mk_g(2, bgav_sb, gbv, b) for b in range(8, NB)]
            fillers_v += [mk_g(3, bgah_sb, gbh, b) for b in range(NB)]

            noop = lambda: None
            rsx0 = rpool.tile([128, 1], F32, tag="rsx0")
            b1_loop(f_v, Av, rs0v, rs1v, rinvv,
                    fold_src=gbv, fold_dst=gTv, fold_lag=6,
                    fillers=[noop] * 3 + fillers_v,
                    direct_q=ffv0, split0=rsx0, tag="v")

            # residual x: one transposing DMA into c-chunk layout
            staged(nc.sync.dma_start(
                out=xr, in_=xrd[:].rearrange("(o p) n -> p o n", p=128)),
                fh_done[-1])

            # B1(h) with B2(v) units as PE fillers
            units_v = []
            ob_box = {}

            def mk_apply_v(jt_i):
                def go():
                    ob_box[jt_i] = apply_unit(gTv, Av, jt_i)
                return go

            def mk_conv_v(jt_i):
                return lambda: outconv_unit(
                    ob_box[jt_i], 0, bfav_sb, ov_t, jt_i)

            for jt_i in range(5):
                units_v.append(mk_apply_v(jt_i))
                units_v.append(mk_conv_v(jt_i))

            b1_loop(f_h, Ah, rs0h, rs1h, rinvh,
                    fold_src=gbh, fold_dst=gTh,
                    fillers=[noop] * 3 + units_v, tag="h")

            # tail: B2(h).  The scores psum pools are free after the last
            # exp, so all 5 applies run concurrently (2 borrowed + 3 ps3);
            # o-copies and residual converts split across ACT and DVE,
            # with 6 c-chunks on the PE-residual + ACT-convert path.
            tail_ap = {}
            tail_pools = {0: sApool, 1: sBpool, 2: ps3, 3: ps3, 4: ps3}
            tail_acts = {0: (), 1: (), 2: (3,), 3: (2, 3), 4: (1, 2, 3)}
            for jt_i in range(5):
                j0, jw = JTS[jt_i]
                pool = tail_pools[jt_i]
                ap = pool.tile([128, 512], F32, tag=pool.name)
                for pi in range(NB // 2):
                    nc.tensor.matmul(
                        ap[:, :jw],
                        lhsT=gTh[:, 2 * pi : 2 * pi + 2, :],
                        rhs=Ah[:, 2 * pi : 2 * pi + 2, j0 : j0 + jw],
                        start=(pi == 0), stop=(pi == NB // 2 - 1),
                        perf_mode=DR)
                tail_ap[jt_i] = ap
            tail_ob = {}
            for jt_i in range(5):
                jw = JTS[jt_i][1]
                ob = obh.tile([128, 512], BF16, tag=f"obt{jt_i}")
                if jt_i % 2 == 0:
                    nc.scalar.activation(
                        out=ob[:, :jw], in_=tail_ap[jt_i][:, :jw],
                        func=COPY, bias=0.0, scale=1.0)
                else:
                    nc.vector.tensor_copy(
                        out=ob[:, :jw], in_=tail_ap[jt_i][:, :jw])
                tail_ob[jt_i] = ob
            # out-convs: psum rotates over 5 slots (3 ps3 + freed sA/sB),
            # residual converts alternate DVE / (PE-residual + ACT)
            cp_pools = [ps3, ps3, ps3, sApool, sBpool]
            idx = 0
            for jt_i in range(5):
                j0, jw = JTS[jt_i]
                ob = tail_ob[jt_i]
                outt = outh.tile([128, CCH, 512], BF16, tag=f"outt{jt_i}")
                for co in range(CCH):
                    pool = cp_pools[idx % 5]
                    on_act = idx % 2 == 1
                    idx += 1
                    cp = pool.tile([128, 512], F32, tag=pool.name)
                    nc.tensor.matmul(
                        cp[:, :jw],
                        lhsT=wpb[:, C + co * 128 : C + co * 128 + 128],
                        rhs=ob[:, :jw], start=True, stop=not on_act)
                    if on_act:
                        nc.tensor.matmul(
                            cp[:, :jw], lhsT=ident_sb,
                            rhs=xr[:, co, j0 : j0 + jw],
                            start=False, stop=True)
                        nc.scalar.activation(
                            out=outt[:, co, :jw], in_=cp[:, :jw],
                            func=IDENT, bias=bfah_sb[:, co : co + 1],
                            scale=1.0)
                    else:
                        nc.vector.scalar_tensor_tensor(
                            out=outt[:, co, :jw], in0=cp[:, :jw],
                            scalar=bfah_sb[:, co : co + 1],
                            in1=xr[:, co, j0 : j0 + jw], op0=ADD, op1=ADD)
                st_eng = nc.sync if True else nc.sync
                st_eng.dma_start(
                    out=oh_t[:, :, j0 : j0 + jw], in_=outt[:, :, :jw])

            if DEBUG_TAPS:
                taps = {
                    "d_ffa": ffa, "d_fa": f_a, "d_fv": f_v, "d_fh": f_h,
                    "d_av0": Av[:, 0, :], "d_av17": Av[:, 17, :],
                    "d_ah17": Ah[:, 17, :],
                    "d_rs0v": rs0v, "d_rinvv": rinvv, "d_rinvh": rinvh,
                    "d_gbv": gbv, "d_gTv": gTv, "d_gTh": gTh,
                }
                for nm, ap in taps.items():
                    sh = [ap.shape[0], int(np.prod(ap.shape[1:]))]
                    dd = nc.declare_dram_parameter(nm, sh, ap.dtype,
                                                   isOutput=True)
                    nc.sync.dma_start(out=dd[:], in_=ap)

    _split_multi_waits(nc)
    return nc


_NC = None


def _get_nc():
    global _NC
    if _NC is None:
        _NC = _build_nc()
    return _NC


def _fold_weights(Wa, ba, ga, ta, Wv, bv, gv, tv, Wgav, bgav, Wgah, bgah,
                  Wfav, bfav, Wfah, bfah):
    s_a = ga / np.sqrt(1.0 + EPS)
    s_v = gv / np.sqrt(1.0 + EPS)
    Wa_f = Wa * s_a[:, None]
    ba_f = ba * s_a + ta
    Wv_f = Wv * s_v[:, None]
    bv_f = bv * s_v + tv

    def wt8(Wm):  # [MID, C] -> [128, CCH*MID] with c = 4p + cch
        return Wm.T.reshape(128, CCH * MID)

    def col_pre(b):  # [C] -> [c % 128, c // 128]
        return b.reshape(CCH, 128).T

    wpack8 = np.concatenate(
        [wt8(Wa_f), wt8(Wv_f), wt8(Wgav), wt8(Wgah)], axis=1)
    wpackb = np.concatenate(
        [Wfav.T / GSC, Wfah.T / GSC, np.eye(128, dtype=np.float32)], axis=1)
    fpack = np.concatenate(
        [ba_f.reshape(MID, 1), bv_f.reshape(MID, 1),
         col_pre(bfav), col_pre(bfah),
         np.broadcast_to(bgav.reshape(1, MID), (128, MID)),
         np.broadcast_to(bgah.reshape(1, MID), (128, MID)),
         np.full((MID, 1), EXPB, np.float32)], axis=1)
    return {
        "wpack8": np.ascontiguousarray(wpack8).astype(NPF8),
        "wpackb": np.ascontiguousarray(wpackb).astype(NPBF),
        "fpack": np.ascontiguousarray(fpack, dtype=np.float32),
    }


def kernel(x, x_h, x_v, Wa, ba, ga, ta, Wv, bv, gv, tv,
           Wgav, bgav, Wgah, bgah, Wfav, bfav, Wfah, bfah):
    x = np.asarray(x, dtype=np.float32)
    x_h = np.asarray(x_h, dtype=np.float32)
    x_v = np.asarray(x_v, dtype=np.float32)
    shared = _fold_weights(
        np.asarray(Wa, np.float32), np.asarray(ba, np.float32),
        np.asarray(ga, np.float32), np.asarray(ta, np.float32),
        np.asarray(Wv, np.float32), np.asarray(bv, np.float32),
        np.asarray(gv, np.float32), np.asarray(tv, np.float32),
        np.asarray(Wgav, np.float32), np.asarray(bgav, np.float32),
        np.asarray(Wgah, np.float32), np.asarray(bgah, np.float32),
        np.asarray(Wfav, np.float32), np.asarray(bfav, np.float32),
        np.asarray(Wfah, np.float32), np.asarray(bfah, np.float32),
    )

    in_maps = []
    for b in range(B):
        xb = np.ascontiguousarray(x[b].reshape(C, N))
        m = dict(shared)
        m["x8"] = xb.astype(NPF8)
        m["xv8"] = np.ascontiguousarray(x_v[b].reshape(C, N)).astype(NPF8)
        m["xh8"] = np.ascontiguousarray(x_h[b].reshape(C, N)).astype(NPF8)
        m["xr"] = xb.astype(NPBF)
        in_maps.append(m)

    nc = _get_nc()
    res = run_bass_kernel_spmd(nc, in_maps, core_ids=list(range(B)))
    o_h = np.stack([res.results[b]["oh"].astype(np.float32).reshape(C, H, W)
                    for b in range(B)])
    o_v = np.stack([res.results[b]["ov"].astype(np.float32).reshape(C, H, W)
                    for b in range(B)])
    return (o_h, o_v)
